# revision 19
# baseline (speedup 1.0000x reference)
"""BCEWithLogitsLoss(mean) over (8192, 8192) logits with binary-step targets,
data-parallel over 8 NeuronCores (1024 rows each).

loss = mean(softplus(x) - x * t),  t[i,j] = 1 if j < targets[i] else 0
     = [ sum softplus(x)  -  sum_{j<t_i} x[i,j] ] / (B*N)

No softplus ACT table exists in this compiler, so softplus is computed as
ln(1 + exp(x)) -- exp and ln live in the same ACT table set.  x ships to
the device as bf16 (host-converted), halving HBM traffic (~5e-5 relative
loss error).  Per-core pipeline, one [128, 8192] row-block tile per step:

  SYNC  dma x row-block (2 MiB bf16) -> SBUF; iota/tlen once at start
  ACT   u = exp(x) (bf16), then ln(1+u) with accum_out -> per-row-block
        softplus sums
  DVE   ONE fused op: scalar_tensor_tensor
            out = (iota < t) * x,  accum_out = per-partition sum
        i.e. the whole masked-sum term in a single instruction per tile

Raw Bass with manual semaphores (the Tile framework's exit drain and all
bass_isa raw-ISA ops are rejected by this environment's compiler build).
Host reduces the tiny [128, 8] partial-sum outputs in float64.
"""

import numpy as np

_B, _N = 8192, 8192
_NCORES = 8
_ROWS = _B // _NCORES  # 1024 rows per core
_P = 128
_RB = _ROWS // _P  # 8 row-block tiles per core
_CH = 2048  # stt column chunk (fp16 iota stays integer-exact below 2048)
_NCH = _N // _CH

_cache = {}


def _build_nc(repeat=1):
    import concourse.bass as bass
    import concourse.mybir as mybir

    f32 = mybir.dt.float32
    bf16 = mybir.dt.bfloat16
    fp16 = mybir.dt.float16
    A = mybir.AluOpType
    F = mybir.ActivationFunctionType

    nc = bass.Bass()
    x_d = nc.dram_tensor("x", [_ROWS, _N], bf16, kind="ExternalInput")
    tlen_d = nc.dram_tensor("tlen", [_P, _RB * _NCH], f32, kind="ExternalInput")
    iota_d = nc.dram_tensor("iota", [_P, _CH], fp16, kind="ExternalInput")
    sp_d = nc.dram_tensor("sp_out", [_P, _RB], f32, kind="ExternalOutput")
    xma_d = nc.dram_tensor("xma_out", [_P, _RB * _NCH], f32, kind="ExternalOutput")

    from contextlib import ExitStack

    with ExitStack() as ctx:
        xt2 = ctx.enter_context(nc.sbuf_tensor([_P, 2 * _N], bf16))  # 2-buf x
        ut2 = ctx.enter_context(nc.sbuf_tensor([_P, 2 * _N], bf16))  # 2-buf exp/ln
        jt2 = ctx.enter_context(nc.sbuf_tensor([_P, 2 * _N], bf16))  # 2-buf stt junk
        wt2 = ctx.enter_context(nc.sbuf_tensor([_P, 2 * 6144], bf16))  # 2-buf combine
        iota_f = ctx.enter_context(nc.sbuf_tensor([_P, _CH], fp16))
        tlen_sb = ctx.enter_context(nc.sbuf_tensor([_P, _RB * _NCH], f32))
        sp_acc = ctx.enter_context(nc.sbuf_tensor([_P, _RB], f32))
        xma_acc = ctx.enter_context(nc.sbuf_tensor([_P, _RB * _NCH], f32))
        dsem0 = ctx.enter_context(nc.semaphore())  # x loads, even tiles
        dsem1 = ctx.enter_context(nc.semaphore())  # x loads, odd tiles
        tsem = ctx.enter_context(nc.semaphore())  # tlen load (+16)
        isem = ctx.enter_context(nc.semaphore())  # iota load (+16)
        asem = ctx.enter_context(nc.semaphore())  # exp completions
        lsem = ctx.enter_context(nc.semaphore())  # ln completions
        vsem = ctx.enter_context(nc.semaphore())  # stt completions
        csem = ctx.enter_context(nc.semaphore())  # combine completions
        fsem = ctx.enter_context(nc.semaphore())  # final out dmas
        block = ctx.enter_context(nc.Block())
        xt = [xt2[:, :_N], xt2[:, _N:]]
        ut = [ut2[:, :_N], ut2[:, _N:]]
        jt = [jt2[:, :_N], jt2[:, _N:]]
        wt = [wt2[:, :6144], wt2[:, 6144:]]

        _T = repeat * _RB

        @block.sync
        def _(sync):
            sync.dma_start(out=tlen_sb[:], in_=tlen_d[:]).then_inc(tsem, 16)
            sync.dma_start(out=iota_f[:], in_=iota_d[:]).then_inc(isem, 16)
            for vt in range(_T):
                rb = vt % _RB
                if vt >= 2:
                    sync.wait_ge(asem, vt - 1)
                    sync.wait_ge(vsem, vt - 1)
                sync.dma_start(
                    out=xt[vt % 2], in_=x_d[rb * _P : (rb + 1) * _P, :]
                ).then_inc(dsem0 if vt % 2 == 0 else dsem1, 16)
            # final outputs
            sync.wait_ge(lsem, _T)
            sync.dma_start(out=sp_d[:], in_=sp_acc[:]).then_inc(fsem, 16)
            sync.wait_ge(vsem, _T)
            sync.dma_start(out=xma_d[:], in_=xma_acc[:]).then_inc(fsem, 16)
            sync.wait_ge(fsem, 32)

        @block.scalar
        def _(scalar):
            def emit_ln(vt):
                scalar.wait_ge(csem, 4 * (vt + 1))
                nc.scalar.activation(
                    wt[vt % 2][:, 4096:6144],
                    wt[vt % 2][:, 4096:6144],
                    F.Ln,
                    bias=1.0,
                    scale=1.0,
                    accum_out=sp_acc[:, (vt % _RB) : (vt % _RB) + 1],
                ).then_inc(lsem, 1)

            for vt in range(_T):
                scalar.wait_ge(dsem0 if vt % 2 == 0 else dsem1, 16 * (vt // 2 + 1))
                if vt >= 2:
                    scalar.wait_ge(csem, 4 * (vt - 1))  # ut[vt%2] freed by combine vt-2
                nc.scalar.activation(ut[vt % 2], xt[vt % 2], F.Exp).then_inc(asem, 1)
                if vt >= 1:
                    emit_ln(vt - 1)  # ln vt-1 runs behind exp vt: no RAW bubble
            emit_ln(_T - 1)

        @block.vector
        def _(vector):
            vector.wait_ge(isem, 16)
            vector.wait_ge(tsem, 16)
            for vt in range(_T):
                rb = vt % _RB
                vector.wait_ge(dsem0 if vt % 2 == 0 else dsem1, 16 * (vt // 2 + 1))
                if vt >= 2:
                    vector.wait_ge(vsem, vt - 1)  # jt[vt%2] freed by stt vt-2
                for ci in range(_NCH):
                    sl = rb * _NCH + ci
                    ins = nc.vector.scalar_tensor_tensor(
                        out=jt[vt % 2][:, ci * _CH : (ci + 1) * _CH],
                        in0=iota_f[:],
                        scalar=tlen_sb[:, sl : sl + 1],
                        in1=xt[vt % 2][:, ci * _CH : (ci + 1) * _CH],
                        op0=A.is_lt,
                        op1=A.mult,
                        accum_out=xma_acc[:, sl : sl + 1],
                    )
                ins.then_inc(vsem, 1)
                # pair/quad combine: w = (1+A)(1+B)-1 built as (A+1)*B + A
                vector.wait_ge(asem, vt + 1)  # exp vt done
                if vt >= 2:
                    vector.wait_ge(lsem, vt - 1)  # wt[vt%2] freed by ln vt-2
                u_ = ut[vt % 2]
                w_ = wt[vt % 2]
                # level 1: halves of u -> w[:, :4096]
                nc.vector.scalar_tensor_tensor(
                    out=w_[:, :4096], in0=u_[:, :4096], scalar=1.0,
                    in1=u_[:, 4096:], op0=A.add, op1=A.mult,
                ).then_inc(csem, 1)
                vector.wait_ge(csem, 4 * vt + 1)
                nc.vector.tensor_tensor(
                    out=w_[:, :4096], in0=w_[:, :4096], in1=u_[:, :4096], op=A.add
                ).then_inc(csem, 1)
                vector.wait_ge(csem, 4 * vt + 2)
                # level 2: halves of w[:, :4096] -> w[:, 4096:6144]
                nc.vector.scalar_tensor_tensor(
                    out=w_[:, 4096:6144], in0=w_[:, :2048], scalar=1.0,
                    in1=w_[:, 2048:4096], op0=A.add, op1=A.mult,
                ).then_inc(csem, 1)
                vector.wait_ge(csem, 4 * vt + 3)
                nc.vector.tensor_tensor(
                    out=w_[:, 4096:6144], in0=w_[:, 4096:6144], in1=w_[:, :2048],
                    op=A.add,
                ).then_inc(csem, 1)

    return nc


def _get_nc():
    if "nc" not in _cache:
        _cache["nc"] = _build_nc()
    return _cache["nc"]


def _prep_in_maps(inputs, targets):
    import ml_dtypes

    x = np.asarray(inputs, dtype=np.float32)
    t = np.asarray(targets).astype(np.float64)  # values < 2**24, exact in f32
    assert x.shape == (_B, _N) and t.shape == (_B,)
    xb = x.astype(ml_dtypes.bfloat16)
    iota = np.ascontiguousarray(
        np.broadcast_to(np.arange(_CH, dtype=np.float16)[None, :], (_P, _CH))
    )
    coff = (np.arange(_NCH, dtype=np.float32) * _CH)[None, :]  # [1, NCH]
    in_maps = []
    for c in range(_NCORES):
        xs = np.ascontiguousarray(xb[c * _ROWS : (c + 1) * _ROWS])
        ts = t[c * _ROWS : (c + 1) * _ROWS]
        # tlen[p, rb*NCH+ci] = targets[c*1024 + rb*128 + p] - 2048*ci
        tl = ts.reshape(_RB, _P).T.astype(np.float32)  # [P, RB]
        tlen = np.ascontiguousarray(
            (tl[:, :, None] - coff[None, :, :]).reshape(_P, _RB * _NCH)
        )
        in_maps.append({"x": xs, "tlen": tlen, "iota": iota})
    return in_maps


def kernel(inputs, targets):
    from concourse.bass_utils import run_bass_kernel_spmd

    nc = _get_nc()
    in_maps = _prep_in_maps(inputs, targets)

    res = run_bass_kernel_spmd(nc, in_maps, list(range(_NCORES)))

    total = np.float64(0.0)
    for c in range(_NCORES):
        total += np.sum(res.results[c]["sp_out"].astype(np.float64))
        total -= np.sum(res.results[c]["xma_out"].astype(np.float64))
    loss = total / (np.float64(_B) * np.float64(_N))
    return np.float32(loss)
